# revision 2
# baseline (speedup 1.0000x reference)
"""Trainium2 Bass kernel for Conv2d(128->256, 3x3, stride 1, pad 1) on
x(32,128,56,56) fp32 — data-parallel over batch across 8 NeuronCores, with
1D Winograd F(2,3) along H (z0 = m0+m1+m2, z1 = m1-m2-m3 per output-row
pair; the G-transform of the kh axis is folded into the weights on the
host, so the device GEMMs contract C_IN=128 over 4 i-positions x 3 kw
shifts = 12 matmul streams per row-pair vs direct conv's 18).

Measured on HW: 115.4us/iter vs 135.3us for the direct-conv baseline.
PE streams 150,528 columns (vs 225,792 direct); fitted HW cost ~0.42ns/col
+ ~42ns/matmul + ~230ns per PSUM bank-group switch (128 groups here).

Hard-won HW facts baked into this structure (CoreSim models NONE of them —
it predicts 70us; always re-verify changes on hardware):
  - An op may read only ONE non-scalar input from PSUM (walrus
    NCC_IBVF027), so the m_i are ACT-evacuated to SBUF f32 before the
    vector engines combine them.
  - gpsimd ("Pool") has no TensorScalarPtr opcode (NCC_IXCG966): only
    plain tensor_tensor there; scalar_tensor_tensor lives on DVE.
  - Unrolling 2 bodies per For_i iteration is ~20% SLOWER on HW (program
    size blows the instruction-fetch budget); keep the body small.
  - Replacing the two DVE scalar_tensor_tensor combines with extra
    solo ACT evacuations + plain adds measured SLOWER (130.8us), as did
    F(4,3) (226us, vector-op-bound on HW despite the best sim time).
  - The moving matmul operand must stay f32r: a bf16 moving operand
    lowers to serialized Ldweights+Matmult pairs.

Per row-group (7 tile-rows x 56 cols = 392 psum columns):
  PE:   4 i x 3 kw matmuls; kw-taps accumulate back-to-back per bank;
        i-pairs share a 2-bank psum tile (512-padded, bank-aligned)
  ACT:  one strided evacuation per i-pair, PSUM -> SBUF f32
  Pool: t0 = mb0+mb1, t1 = mb1-mb2 (plain adds)
  DVE:  z0 = (mb2+b)+t0, z1 = (t1+b)-mb3 (scalar_tensor_tensor folds the
        bias), bf16 out; plus input-transform ops V0/V2 (V1/V3 on Pool)
  SP:   x loads and per-row-group output stores
Output rows leave parity-split (z0=even rows, z1=odd); the host
re-interleaves and upcasts (absmax rel err ~2.1e-3, gate is 2e-2).
"""
import numpy as np

N_CORES = 8
N_PER_CORE = 4
C_IN, C_OUT, K = 128, 256, 3
H = W = 56
HP = WP = 58
TR = H // 2           # 28 tile-rows (output-row pairs)
TRG = 7               # tile-rows per matmul group
N_RG = TR // TRG      # 4 groups
NFREE = TRG * W       # 392 psum columns per group

_compiled = {}


def _patch_ldw_opt():
    import concourse.bass_utils as bu

    if getattr(bu.run_command, "_ldw_patched", False):
        return
    orig = bu.run_command

    def patched(argv, **kw):
        import os
        argv = ["--enable-ldw-opt=true" if a == "--enable-ldw-opt=false" else a
                for a in argv]
        pol = os.environ.get("BASS_WALRUS_POLICY")
        if pol:
            argv = [f"--policy={pol}" if a == "--policy=0" else a
                    for a in argv]
        extra = os.environ.get("BASS_WALRUS_EXTRA")
        if extra and argv and "walrus_driver" in str(argv[0]):
            argv = list(argv) + extra.split()
        return orig(argv, **kw)

    patched._ldw_patched = True
    bu.run_command = patched


def _build(reps: int | None = None):
    import concourse.bass as bass  # noqa: F401
    import concourse.mybir as mybir
    import concourse.tile as tile
    from concourse import bacc

    _patch_ldw_opt()

    f32 = mybir.dt.float32
    f32r = mybir.dt.float32r
    bf16 = mybir.dt.bfloat16
    AF = mybir.ActivationFunctionType

    nc = bacc.Bacc("TRN2", target_bir_lowering=False, debug=False,
                   num_devices=N_CORES)
    x_d = nc.declare_dram_parameter("x", [N_PER_CORE, C_IN, HP * WP], f32r,
                                    isOutput=False)
    u_d = nc.declare_dram_parameter("u", [C_IN, 4 * K * 2 * 128], f32r,
                                    isOutput=False)
    b_d = nc.declare_dram_parameter("b", [128, 4], f32, isOutput=False)
    o_d = nc.declare_dram_parameter("o", [N_PER_CORE, 2, 128, 2 * TR * W],
                                    bf16, isOutput=True)

    with tile.TileContext(nc) as tc:
        with (
            tc.tile_pool(name="const", bufs=1) as const_pool,
            tc.tile_pool(name="xp", bufs=3) as x_pool,
            tc.tile_pool(name="vp", bufs=2) as v_pool,
            tc.tile_pool(name="mp", bufs=2) as m_pool,
            tc.tile_pool(name="tp", bufs=4) as t_pool,
            tc.tile_pool(name="op", bufs=2) as o_pool,
            tc.tile_pool(name="ps", bufs=4, space="PSUM") as psum_pool,
        ):
            b_sb = const_pool.tile([128, 4], f32, tag="b")
            u_sb = const_pool.tile([C_IN, 4 * K * 2 * 128], f32r, tag="u")
            x_first = x_pool.tile([C_IN, HP * WP], f32r, tag="x")
            nc.sync.dma_start(u_sb[:, 0:256], u_d[:, 0:256])
            nc.sync.dma_start(x_first[:], x_d[0])
            nc.sync.dma_start(b_sb[:], b_d[:])
            nc.sync.dma_start(u_sb[:, 256:], u_d[:, 256:])
            # warm the ACT function table once so LoadActFuncSet doesn't
            # stall each For_i iteration
            scrap = const_pool.tile([128, 4], f32, tag="warm")
            nc.scalar.activation(scrap[:], b_sb[:], AF.Identity, bias=0.0,
                                 scale=1.0)

            TT = mybir.AluOpType

            def transform(x_tile, name):
                """Input transform V_i over row windows [2tr .. 2tr+3].

                V0/V2 on DVE, V1/V3 on Pool: the planes land pairwise in
                parallel, so the PE (which consumes V_i in i-order, 3
                matmuls each) is never gated on a single engine's queue.
                """
                xv = x_tile[:].rearrange("p (h w) -> p h w", w=WP)
                v_sb = v_pool.tile([C_IN, 4 * TR * WP], f32r, tag="v",
                                   name=name)
                v4 = v_sb[:].rearrange("p (i t w) -> p i t w", i=4, w=WP)

                def xrow(k):
                    # rows k, k+2, ..., k+54 (28 rows stride 2), all cols
                    return xv[:, k:k + 2 * TR - 1:2, :]

                nc.vector.tensor_tensor(v4[:, 0], xrow(0), xrow(2),
                                        TT.subtract)
                nc.gpsimd.tensor_tensor(v4[:, 1], xrow(1), xrow(2),
                                        TT.add)
                nc.vector.tensor_tensor(v4[:, 2], xrow(2), xrow(1),
                                        TT.subtract)
                nc.gpsimd.tensor_tensor(v4[:, 3], xrow(1), xrow(3),
                                        TT.subtract)
                return v4

            def body():
                x_tiles = [x_first]
                for n in range(1, N_PER_CORE):
                    x_sb = x_pool.tile([C_IN, HP * WP], f32r, tag="x",
                                       name=f"x{n}")
                    nc.sync.dma_start(x_sb[:], x_d[n])
                    x_tiles.append(x_sb)

                v4 = transform(x_first, "v0")
                for n in range(N_PER_CORE):
                    for cob in range(2):
                        o_sb = o_pool.tile([128, 2 * TR * W], bf16, tag="o")
                        for rg in range(N_RG):
                            # m_i pairs in 2-bank psum tiles, 512-padded so
                            # each 392-col group is bank-aligned; each pair's
                            # single-op evac starts when its 6 matmuls stop
                            mb = m_pool.tile([128, 4 * NFREE], f32,
                                             tag="mb")
                            mb4 = mb[:].rearrange("p (i f) -> p i f", i=4)
                            for hf in range(2):
                                ps = psum_pool.tile([128, 2 * 512], f32,
                                                    tag="ps")
                                for ii in range(2):
                                    i = hf * 2 + ii
                                    dst = ps[:, ii * 512:ii * 512 + NFREE]
                                    for kw in range(K):
                                        mv = v4[:, i,
                                                rg * TRG:(rg + 1) * TRG,
                                                kw:kw + W]
                                        c0 = ((i * K + kw) * 2 + cob) * 128
                                        nc.tensor.matmul(
                                            dst, u_sb[:, c0:c0 + 128], mv,
                                            start=(kw == 0),
                                            stop=(kw == K - 1),
                                        )
                                ps2 = ps[:].rearrange("p (i f) -> p i f",
                                                      i=2)
                                nc.scalar.activation(
                                    mb4[:, 2 * hf:2 * hf + 2, 0:NFREE],
                                    ps2[:, :, 0:NFREE],
                                    AF.Identity, bias=0.0, scale=1.0)
                            z0 = o_sb[:, rg * 2 * NFREE:
                                      rg * 2 * NFREE + NFREE]
                            z1 = o_sb[:, rg * 2 * NFREE + NFREE:
                                      (rg + 1) * 2 * NFREE]
                            # SBUF-only combines; bias folded into the two
                            # stt ops: z0 = (m2+b)+t0, z1 = (a+b)-m3
                            t0 = t_pool.tile([128, NFREE], f32, tag="t0")
                            nc.gpsimd.tensor_tensor(t0[:], mb4[:, 0],
                                                    mb4[:, 1], TT.add)
                            t1 = t_pool.tile([128, NFREE], f32, tag="t1")
                            nc.gpsimd.tensor_tensor(t1[:], mb4[:, 1],
                                                    mb4[:, 2], TT.subtract)
                            nc.vector.scalar_tensor_tensor(
                                z0, mb4[:, 2], b_sb[:, cob:cob + 1],
                                t0[:], TT.add, TT.add)
                            nc.vector.scalar_tensor_tensor(
                                z1, t1[:], b_sb[:, cob:cob + 1],
                                mb4[:, 3], TT.add, TT.subtract)
                            # store each rg's [z0|z1] as soon as it's done:
                            # short drain tail, early DMA-ring issue
                            nc.sync.dma_start(
                                o_d[n, cob][:, rg * 2 * NFREE:
                                            (rg + 1) * 2 * NFREE],
                                o_sb[:, rg * 2 * NFREE:
                                     (rg + 1) * 2 * NFREE])
                            if cob == 0 and rg == 0 and n + 1 < N_PER_CORE:
                                # hoist next image's input transform so the
                                # PE never waits on V at image boundaries
                                next_v4 = transform(x_tiles[n + 1],
                                                    f"v{n + 1}")
                    if n + 1 < N_PER_CORE:
                        v4 = next_v4

            if reps is None:
                body()
            else:
                with tc.For_i(0, reps, 1):
                    body()

    nc.compile()
    return nc


def _prep_inputs(x, weight, bias):
    x = np.asarray(x, dtype=np.float32)
    weight = np.asarray(weight, dtype=np.float32)
    bias = np.asarray(bias, dtype=np.float32)

    xp = np.pad(x, ((0, 0), (0, 0), (1, 1), (1, 1)))          # (32,128,58,58)
    xp = xp.reshape(N_CORES, N_PER_CORE, C_IN, HP * WP)

    # weight (co, ci, kh, kw) -> U[i, kw, cob, co_128, ci] via G over kh
    wg = weight.reshape(2, 128, C_IN, K, K)                    # cob,co,ci,kh,kw
    g0, g1, g2 = wg[..., 0, :], wg[..., 1, :], wg[..., 2, :]   # cob,co,ci,kw
    u = np.stack([g0, (g0 + g1 + g2) * 0.5, (g0 - g1 + g2) * 0.5, g2])
    # u: [i, cob, co, ci, kw] -> [ci, i, kw, cob, co]
    u = u.transpose(3, 0, 4, 1, 2)
    u = np.ascontiguousarray(u).reshape(C_IN, 4 * K * 2 * 128)

    br = np.ascontiguousarray(bias.reshape(2, 128).T)          # [128, 2]
    b4 = np.concatenate([br, -br], axis=1)                     # [128, 4]

    return [
        {"x": np.ascontiguousarray(xp[c]), "u": u, "b": b4}
        for c in range(N_CORES)
    ]


def _postprocess(o_np):
    """o (n_cores, 4, 2, 128, 2*28*56) bf16 -> (32, 256, 56, 56) f32."""
    o = np.asarray(o_np, dtype=np.float32)
    o = o.reshape(-1, 2, 128, N_RG, 2, TRG, W)
    # [n, cob, co, rg, parity, t, w] -> [n, cob, co, rg, t, parity, w]
    o = o.transpose(0, 1, 2, 3, 5, 4, 6)
    return np.ascontiguousarray(o).reshape(-1, C_OUT, H, W)


def kernel(x: np.ndarray, weight: np.ndarray, bias: np.ndarray) -> np.ndarray:
    from concourse.bass_utils import run_bass_kernel_spmd

    if "nc" not in _compiled:
        _compiled["nc"] = _build()
    nc = _compiled["nc"]

    in_maps = _prep_inputs(x, weight, bias)
    res = run_bass_kernel_spmd(nc, in_maps, list(range(N_CORES)))
    out = np.stack([np.asarray(r["o"]) for r in res.results])
    return _postprocess(out)


# revision 3
# speedup vs baseline: 1.1097x; 1.1097x over previous
"""Trainium2 Bass kernel for Conv2d(128->256, 3x3, stride 1, pad 1) on
x(32,128,56,56) fp32 — data-parallel over batch across 8 NeuronCores, with
1D Winograd F(2,3) along H (z0 = m0+m1+m2, z1 = m1-m2-m3 per output-row
pair; the G-transform of the kh axis is folded into the weights on the
host, so the device GEMMs contract C_IN=128 over 4 i-positions x 3 kw
shifts = 12 matmul streams per row-pair vs direct conv's 18).

Measured on HW: 115.4us/iter vs 135.3us for the direct-conv baseline.
PE streams 150,528 columns (vs 225,792 direct); fitted HW cost ~0.42ns/col
+ ~42ns/matmul + ~230ns per PSUM bank-group switch (128 groups here).

Hard-won HW facts baked into this structure (CoreSim models NONE of them —
it predicts 70us; always re-verify changes on hardware):
  - An op may read only ONE non-scalar input from PSUM (walrus
    NCC_IBVF027), so the m_i are ACT-evacuated to SBUF f32 before the
    vector engines combine them.
  - gpsimd ("Pool") has no TensorScalarPtr opcode (NCC_IXCG966): only
    plain tensor_tensor there; scalar_tensor_tensor lives on DVE.
  - Unrolling 2 bodies per For_i iteration is ~20% SLOWER on HW (program
    size blows the instruction-fetch budget); keep the body small.
  - Replacing the two DVE scalar_tensor_tensor combines with extra
    solo ACT evacuations + plain adds measured SLOWER (130.8us), as did
    F(4,3) (226us, vector-op-bound on HW despite the best sim time) and
    merging the two per-group pair-evacuations into one 4-bank ACT op
    (121.0us: halving ACT ops does not pay for the PSUM-recycle stall).
    Full HW ladder: direct 135.3 / this 115.4 / pair-wide-combines 115.8 /
    merged-evac 121.0 / no-stt 130.8 / unroll-2 139.0 / F(4,3) 226.1.
  - The moving matmul operand must stay f32r: a bf16 moving operand
    lowers to serialized Ldweights+Matmult pairs.

Per row-group (7 tile-rows x 56 cols = 392 psum columns):
  PE:   4 i x 3 kw matmuls; kw-taps accumulate back-to-back per bank;
        i-pairs share a 2-bank psum tile (512-padded, bank-aligned)
  ACT:  one strided evacuation per i-pair, PSUM -> SBUF f32
  Pool: t0 = mb0+mb1, t1 = mb1-mb2 (plain adds)
  DVE:  z0 = (mb2+b)+t0, z1 = (t1+b)-mb3 (scalar_tensor_tensor folds the
        bias), bf16 out; plus input-transform ops V0/V2 (V1/V3 on Pool)
  SP:   x loads and per-row-group output stores
Output rows leave parity-split (z0=even rows, z1=odd); the host
re-interleaves and upcasts (absmax rel err ~2.1e-3, gate is 2e-2).
"""
import numpy as np

N_CORES = 8
N_PER_CORE = 4
C_IN, C_OUT, K = 128, 256, 3
H = W = 56
HP = WP = 58
TR = H // 2           # 28 tile-rows (output-row pairs)
TRG = 7               # tile-rows per matmul group
N_RG = TR // TRG      # 4 groups
NFREE = TRG * W       # 392 psum columns per group

_compiled = {}


def _patch_ldw_opt():
    import concourse.bass_utils as bu

    if getattr(bu.run_command, "_ldw_patched", False):
        return
    orig = bu.run_command

    def patched(argv, **kw):
        import os
        argv = ["--enable-ldw-opt=true" if a == "--enable-ldw-opt=false" else a
                for a in argv]
        pol = os.environ.get("BASS_WALRUS_POLICY")
        if pol:
            argv = [f"--policy={pol}" if a == "--policy=0" else a
                    for a in argv]
        extra = os.environ.get("BASS_WALRUS_EXTRA")
        if extra and argv and "walrus_driver" in str(argv[0]):
            argv = list(argv) + extra.split()
        return orig(argv, **kw)

    patched._ldw_patched = True
    bu.run_command = patched


def _build(reps: int | None = None):
    import concourse.bass as bass  # noqa: F401
    import concourse.mybir as mybir
    import concourse.tile as tile
    from concourse import bacc

    _patch_ldw_opt()

    f32 = mybir.dt.float32
    f32r = mybir.dt.float32r
    bf16 = mybir.dt.bfloat16
    AF = mybir.ActivationFunctionType

    nc = bacc.Bacc("TRN2", target_bir_lowering=False, debug=False,
                   num_devices=N_CORES)
    x_d = nc.declare_dram_parameter("x", [N_PER_CORE, C_IN, HP * WP], f32r,
                                    isOutput=False)
    u_d = nc.declare_dram_parameter("u", [C_IN, 4 * K * 2 * 128], f32r,
                                    isOutput=False)
    b_d = nc.declare_dram_parameter("b", [128, 4], f32, isOutput=False)
    o_d = nc.declare_dram_parameter("o", [N_PER_CORE, 2, 128, 2 * TR * W],
                                    bf16, isOutput=True)

    with tile.TileContext(nc) as tc:
        with (
            tc.tile_pool(name="const", bufs=1) as const_pool,
            tc.tile_pool(name="xp", bufs=3) as x_pool,
            tc.tile_pool(name="vp", bufs=2) as v_pool,
            tc.tile_pool(name="mp", bufs=2) as m_pool,
            tc.tile_pool(name="tp", bufs=4) as t_pool,
            tc.tile_pool(name="op", bufs=2) as o_pool,
            tc.tile_pool(name="ps", bufs=4, space="PSUM") as psum_pool,
        ):
            b_sb = const_pool.tile([128, 4], f32, tag="b")
            u_sb = const_pool.tile([C_IN, 4 * K * 2 * 128], f32r, tag="u")
            x_first = x_pool.tile([C_IN, HP * WP], f32r, tag="x")
            nc.sync.dma_start(u_sb[:, 0:256], u_d[:, 0:256])
            nc.sync.dma_start(x_first[:], x_d[0])
            nc.sync.dma_start(b_sb[:], b_d[:])
            nc.sync.dma_start(u_sb[:, 256:], u_d[:, 256:])
            # warm the ACT function table once so LoadActFuncSet doesn't
            # stall each For_i iteration
            scrap = const_pool.tile([128, 4], f32, tag="warm")
            nc.scalar.activation(scrap[:], b_sb[:], AF.Identity, bias=0.0,
                                 scale=1.0)

            TT = mybir.AluOpType

            def transform(x_tile, name):
                """Input transform V_i over row windows [2tr .. 2tr+3].

                V0/V2 on DVE, V1/V3 on Pool: the planes land pairwise in
                parallel, so the PE (which consumes V_i in i-order, 3
                matmuls each) is never gated on a single engine's queue.
                """
                xv = x_tile[:].rearrange("p (h w) -> p h w", w=WP)
                v_sb = v_pool.tile([C_IN, 4 * TR * WP], f32r, tag="v",
                                   name=name)
                v4 = v_sb[:].rearrange("p (i t w) -> p i t w", i=4, w=WP)

                def xrow(k):
                    # rows k, k+2, ..., k+54 (28 rows stride 2), all cols
                    return xv[:, k:k + 2 * TR - 1:2, :]

                nc.vector.tensor_tensor(v4[:, 0], xrow(0), xrow(2),
                                        TT.subtract)
                nc.gpsimd.tensor_tensor(v4[:, 1], xrow(1), xrow(2),
                                        TT.add)
                nc.vector.tensor_tensor(v4[:, 2], xrow(2), xrow(1),
                                        TT.subtract)
                nc.gpsimd.tensor_tensor(v4[:, 3], xrow(1), xrow(3),
                                        TT.subtract)
                return v4

            def body():
                x_tiles = [x_first]
                for n in range(1, N_PER_CORE):
                    x_sb = x_pool.tile([C_IN, HP * WP], f32r, tag="x",
                                       name=f"x{n}")
                    nc.sync.dma_start(x_sb[:], x_d[n])
                    x_tiles.append(x_sb)

                v4 = transform(x_first, "v0")
                for n in range(N_PER_CORE):
                    for cob in range(2):
                        o_sb = o_pool.tile([128, 2 * TR * W], bf16, tag="o")
                        for rg in range(N_RG):
                            # m_i pairs in 2-bank psum tiles, 512-padded so
                            # each 392-col group is bank-aligned; each pair's
                            # single-op evac starts when its 6 matmuls stop
                            mb = m_pool.tile([128, 4 * NFREE], f32,
                                             tag="mb")
                            mb4 = mb[:].rearrange("p (i f) -> p i f", i=4)
                            for hf in range(2):
                                ps = psum_pool.tile([128, 2 * 512], f32,
                                                    tag="ps")
                                for ii in range(2):
                                    i = hf * 2 + ii
                                    dst = ps[:, ii * 512:ii * 512 + NFREE]
                                    for kw in range(K):
                                        mv = v4[:, i,
                                                rg * TRG:(rg + 1) * TRG,
                                                kw:kw + W]
                                        c0 = ((i * K + kw) * 2 + cob) * 128
                                        nc.tensor.matmul(
                                            dst, u_sb[:, c0:c0 + 128], mv,
                                            start=(kw == 0),
                                            stop=(kw == K - 1),
                                        )
                                ps2 = ps[:].rearrange("p (i f) -> p i f",
                                                      i=2)
                                nc.scalar.activation(
                                    mb4[:, 2 * hf:2 * hf + 2, 0:NFREE],
                                    ps2[:, :, 0:NFREE],
                                    AF.Identity, bias=0.0, scale=1.0)
                            z0 = o_sb[:, rg * 2 * NFREE:
                                      rg * 2 * NFREE + NFREE]
                            z1 = o_sb[:, rg * 2 * NFREE + NFREE:
                                      (rg + 1) * 2 * NFREE]
                            # SBUF-only combines; bias folded into the two
                            # stt ops: z0 = (m2+b)+t0, z1 = (a+b)-m3
                            t0 = t_pool.tile([128, NFREE], f32, tag="t0")
                            nc.gpsimd.tensor_tensor(t0[:], mb4[:, 0],
                                                    mb4[:, 1], TT.add)
                            t1 = t_pool.tile([128, NFREE], f32, tag="t1")
                            nc.gpsimd.tensor_tensor(t1[:], mb4[:, 1],
                                                    mb4[:, 2], TT.subtract)
                            nc.vector.scalar_tensor_tensor(
                                z0, mb4[:, 2], b_sb[:, cob:cob + 1],
                                t0[:], TT.add, TT.add)
                            nc.vector.scalar_tensor_tensor(
                                z1, t1[:], b_sb[:, cob:cob + 1],
                                mb4[:, 3], TT.add, TT.subtract)
                            # store each rg's [z0|z1] as soon as it's done:
                            # short drain tail, early DMA-ring issue
                            nc.sync.dma_start(
                                o_d[n, cob][:, rg * 2 * NFREE:
                                            (rg + 1) * 2 * NFREE],
                                o_sb[:, rg * 2 * NFREE:
                                     (rg + 1) * 2 * NFREE])
                            if cob == 0 and rg == 0 and n + 1 < N_PER_CORE:
                                # hoist next image's input transform so the
                                # PE never waits on V at image boundaries
                                next_v4 = transform(x_tiles[n + 1],
                                                    f"v{n + 1}")
                    if n + 1 < N_PER_CORE:
                        v4 = next_v4

            if reps is None:
                body()
            else:
                with tc.For_i(0, reps, 1):
                    body()

    nc.compile()
    return nc


def _prep_inputs(x, weight, bias):
    x = np.asarray(x, dtype=np.float32)
    weight = np.asarray(weight, dtype=np.float32)
    bias = np.asarray(bias, dtype=np.float32)

    xp = np.pad(x, ((0, 0), (0, 0), (1, 1), (1, 1)))          # (32,128,58,58)
    xp = xp.reshape(N_CORES, N_PER_CORE, C_IN, HP * WP)

    # weight (co, ci, kh, kw) -> U[i, kw, cob, co_128, ci] via G over kh
    wg = weight.reshape(2, 128, C_IN, K, K)                    # cob,co,ci,kh,kw
    g0, g1, g2 = wg[..., 0, :], wg[..., 1, :], wg[..., 2, :]   # cob,co,ci,kw
    u = np.stack([g0, (g0 + g1 + g2) * 0.5, (g0 - g1 + g2) * 0.5, g2])
    # u: [i, cob, co, ci, kw] -> [ci, i, kw, cob, co]
    u = u.transpose(3, 0, 4, 1, 2)
    u = np.ascontiguousarray(u).reshape(C_IN, 4 * K * 2 * 128)

    br = np.ascontiguousarray(bias.reshape(2, 128).T)          # [128, 2]
    b4 = np.concatenate([br, -br], axis=1)                     # [128, 4]

    return [
        {"x": np.ascontiguousarray(xp[c]), "u": u, "b": b4}
        for c in range(N_CORES)
    ]


def _postprocess(o_np):
    """o (n_cores, 4, 2, 128, 2*28*56) bf16 -> (32, 256, 56, 56) f32."""
    o = np.asarray(o_np, dtype=np.float32)
    o = o.reshape(-1, 2, 128, N_RG, 2, TRG, W)
    # [n, cob, co, rg, parity, t, w] -> [n, cob, co, rg, t, parity, w]
    o = o.transpose(0, 1, 2, 3, 5, 4, 6)
    return np.ascontiguousarray(o).reshape(-1, C_OUT, H, W)


def kernel(x: np.ndarray, weight: np.ndarray, bias: np.ndarray) -> np.ndarray:
    from concourse.bass_utils import run_bass_kernel_spmd

    if "nc" not in _compiled:
        _compiled["nc"] = _build()
    nc = _compiled["nc"]

    in_maps = _prep_inputs(x, weight, bias)
    res = run_bass_kernel_spmd(nc, in_maps, list(range(N_CORES)))
    out = np.stack([np.asarray(r["o"]) for r in res.results])
    return _postprocess(out)
